# revision 1
# baseline (speedup 1.0000x reference)
"""Trainium2 Bass kernel for causal self-attention (B=4, T=2048, C=2048, H=16).

Sharding: 16 heads across 8 cores (2 heads/core), tensor-parallel column split
of Wqkv and row split of Wout; host sums the 8 row-parallel partial outputs.

Device-side layout strategy (everything "transposed", token index in the free
dimension), which makes every matmul land in its natural layout with zero
on-chip transposes:
  qT/kT   [D=128 part, T free]   = Wq_h^T @ x  (m-tile of the qkv projection)
  V       [T part, D free]       computed with x-blocks as the stationary operand
  S^T     [k part, q free]       = kT-block^T-contraction (lhsT=kT blk, rhs=qT)
  U = exp(S^T), causal blocks skipped entirely, diagonal blocks masked via a
      triangular 0/1 multiply after exp
  y^T     [D part, q free]       = sum_kb V_blk.T @ U_blk  (PSUM accumulate)
  denom   [128, q]  Uacc accumulated on DVE (bf16), summed across partitions
          with one gpsimd.partition_all_reduce per q-chunk (no PE matmuls,
          no partition_broadcast: the all-reduce output is already fanned out)
  y_norm  = y^T * reciprocal(Dall)  (DVE reciprocal_approx_fast on [128,CH])
  partial [tokens part, Cout]    lhsT = y^T block, rhs = Wout rows, emitted
            with a one-chunk lag so PE never waits on the softmax
            normalization chain; junk warm-up matmuls cover the initial DMA
            wave so HAM runs the PE at 2.4 GHz from the first real matmul

The attention loop is k-block-outer: for each k-tile, all live q-chunks issue
their S^T and y matmuls back-to-back against the same krot/vsb stationary, so
the PE re-loads each stationary once per k-tile (not once per chunk) and
streams 4x512 columns between weight swaps.

RoPE is applied in [D, T] layout: the q/k columns of Wqkv are permuted on the
host so rotation pairs land at partitions (i, i+64) ("deinterleaved"); the
half-swap is two 64-partition DVE copies, then 3 elementwise ops against
host-precomputed [128, T] cos/sin tables (the 1/sqrt(D) scale is folded into
the q tables). Scores are invariant to any fixed permutation of head dims
applied to both q and k, so the reference is reproduced exactly.
"""

import math

import numpy as np
import ml_dtypes

# Module-level knobs (test harness may set TRACE=True to capture a profile).
TRACE = False
LAST_RESULT = None  # BassKernelResults of the last run (for profiling)

_B, _T, _C, _H = 4, 2048, 2048, 16
_D = 128
_CH = 512  # free-dim chunk (one PSUM bank of fp32)


def build_program(B, T, C, COUT, HPC, n_cores=8):
    """Build the per-core Bass/Tile program (identical on all cores)."""
    import concourse.bass as bass  # noqa: F401
    import concourse.mybir as mybir
    import concourse.tile as tile
    from concourse import bacc
    from concourse import bass_isa
    from contextlib import ExitStack

    dt = mybir.dt
    f32, bf16 = dt.float32, dt.bfloat16
    D, CH = _D, _CH
    KB = C // 128          # contraction blocks for the projections
    NCH = T // CH          # 512-token chunks per batch (also q-chunks)
    NTT = T // 128         # 128-token tiles per batch (also k-tiles)
    BT = B * T
    F = HPC * D            # per-core head feature width
    AF = mybir.ActivationFunctionType

    nc = bacc.Bacc(
        "TRN2", target_bir_lowering=False, debug=False, num_devices=n_cores
    )

    # All weight/activation inputs are pre-tiled on the host so that each
    # SBUF partition's slice is one fully contiguous DRAM run: the DMA then
    # moves 8-16KB descriptors instead of 0.5-1KB ones (~8x faster loads).
    xt = nc.dram_tensor("xt", [128, B * NCH * KB * CH], bf16, kind="ExternalInput").ap()
    wq = nc.dram_tensor("wq", [128, KB * F], bf16, kind="ExternalInput").ap()
    wk = nc.dram_tensor("wk", [128, KB * F], bf16, kind="ExternalInput").ap()
    wv = nc.dram_tensor("wv", [128, KB * F], bf16, kind="ExternalInput").ap()
    wo = nc.dram_tensor("wo", [128, HPC * COUT], bf16, kind="ExternalInput").ap()
    ccq = nc.dram_tensor("ccq", [128, T], bf16, kind="ExternalInput").ap()
    ssq = nc.dram_tensor("ssq", [128, T], bf16, kind="ExternalInput").ap()
    cck = nc.dram_tensor("cck", [128, T], bf16, kind="ExternalInput").ap()
    ssk = nc.dram_tensor("ssk", [128, T], bf16, kind="ExternalInput").ap()
    triu = nc.dram_tensor("triu", [128, 128], bf16, kind="ExternalInput").ap()
    onesb = nc.dram_tensor("onesb", [128, 1], bf16, kind="ExternalInput").ap()
    part = nc.dram_tensor("part", [BT, COUT], bf16, kind="ExternalOutput").ap()

    with ExitStack() as ctx:
        tc = ctx.enter_context(tile.TileContext(nc))
        const_pool = ctx.enter_context(tc.tile_pool(name="const", bufs=1))
        xc_pool = ctx.enter_context(tc.tile_pool(name="xc", bufs=2))
        rot_pool = ctx.enter_context(tc.tile_pool(name="rot", bufs=4 * HPC))
        swap_pool = ctx.enter_context(tc.tile_pool(name="swap", bufs=3))
        a_pool = ctx.enter_context(tc.tile_pool(name="ropeA", bufs=3))
        v_pool = ctx.enter_context(tc.tile_pool(name="vsb", bufs=2))
        u_pool = ctx.enter_context(tc.tile_pool(name="u", bufs=12))
        ua_pool = ctx.enter_context(tc.tile_pool(name="uacc", bufs=6))
        r_pool = ctx.enter_context(tc.tile_pool(name="recip", bufs=4))
        rr_pool = ctx.enter_context(tc.tile_pool(name="rr", bufs=3))
        y_pool = ctx.enter_context(tc.tile_pool(name="y", bufs=2 * HPC))
        o_pool = ctx.enter_context(tc.tile_pool(name="osb", bufs=3))
        # PSUM banks (8 total), shared across phases:
        #   ps_a (2): qk-projection chains in proj phase, S^T in attention
        #   ps_b (4): V projection (1 live) in proj phase, psy_j (4 live) in attn
        #   ps_c (2): out-projection accumulators (interleaved with both phases)
        ps_a = ctx.enter_context(tc.tile_pool(name="psa", bufs=2, space="PSUM"))
        ps_b = ctx.enter_context(tc.tile_pool(name="psb", bufs=4, space="PSUM"))
        ps_c = ctx.enter_context(tc.tile_pool(name="psc", bufs=2, space="PSUM"))

        # ---- persistent constants ----
        # The preload is split across the two hardware DGE queues (sync +
        # scalar) so the critical first-chain inputs (wq, xc0, wk) finish in
        # ~11us instead of waiting behind the full ~8MB preload on one queue.
        wq_sb = const_pool.tile([128, KB * F], bf16, tag="wq")
        wk_sb = const_pool.tile([128, KB * F], bf16, tag="wk")
        xc0 = xc_pool.tile([128, KB * CH], bf16, tag="xc", name="xc0")
        nc.sync.dma_start(wq_sb[:], wq)
        nc.scalar.dma_start(xc0[:], xt[:, 0 : KB * CH])
        nc.sync.dma_start(wk_sb[:], wk)
        ccq_sb = const_pool.tile([128, T], bf16, tag="ccq")
        ssq_sb = const_pool.tile([128, T], bf16, tag="ssq")
        cck_sb = const_pool.tile([128, T], bf16, tag="cck")
        ssk_sb = const_pool.tile([128, T], bf16, tag="ssk")
        for csb, cdr in ((ccq_sb, ccq), (ssq_sb, ssq)):
            nc.sync.dma_start(csb[:], cdr)
        for csb, cdr in ((cck_sb, cck), (ssk_sb, ssk)):
            nc.scalar.dma_start(csb[:], cdr)
        wv_sb = const_pool.tile([128, KB * F], bf16, tag="wv")
        nc.scalar.dma_start(wv_sb[:], wv)
        triu_sb = const_pool.tile([128, 128], bf16, tag="triu")
        nc.sync.dma_start(triu_sb[:], triu)
        onesb_sb = const_pool.tile([128, 1], bf16, tag="onesb")
        nc.sync.dma_start(onesb_sb[:], onesb)
        wo_sb = const_pool.tile([128, HPC * COUT], bf16, tag="wo")
        nc.sync.dma_start(wo_sb[:], wo)

        # ---- PE warm-up: keep the HAM clock gate busy while the initial
        # DMA wave streams in, so the first real matmuls run at 2.4 GHz ----
        junk = const_pool.tile([128, 128], bf16, tag="warmjunk")
        nc.vector.memset(junk[:], 0)
        psw = ps_a.tile([128, 128], f32, tag="psa", name="pswarm")
        for _ in range(120):
            nc.tensor.matmul(
                psw[:], junk[:], junk[:], start=True, stop=True,
                skip_group_check=True,
            )

        # Out-projection is decomposed into micro-units (one m-tile half:
        # 2 wo-chunks x HPC matmuls = 4x512 streamed columns) interleaved
        # into the attention stream at per-k-block granularity.  The PE is
        # in-order, and attention is locally ACT-exp-bound: without filler
        # the PE would stall on exp results; the outproj units depend only
        # on already-normalized chunks, so they absorb that slack.
        # `staging` -> `unitq` promotion is delayed by one finalize point so
        # a chunk's normalize chain (gpsimd + recip + mul) never sits
        # directly in front of its first outproj matmul.
        unitq = []
        staging = []
        osb_live = {}  # (b, j, m) -> osb tile awaiting its second half
        normq = []  # deferred (Dall, psy, ysb_slice) recip+mul, lagged 1 finalize

        def emit_unit():
            if not unitq:
                return False
            ysb_, b_, j_, m, g = unitq.pop(0)
            key = (b_, j_, m)
            if g == 0:
                osb = o_pool.tile([128, COUT], bf16, tag="osb", name="osb")
                osb_live[key] = osb
            else:
                osb = osb_live.pop(key)
            psos = [
                ps_c.tile([128, CH], f32, tag="psc", name="pso")
                for _ in range(2)
            ]
            for h in range(HPC):
                for gi in range(2):
                    nch = g * 2 + gi
                    nc.tensor.matmul(
                        psos[gi][:],
                        ysb_[h][:, m * 128 : (m + 1) * 128],
                        wo_sb[
                            :,
                            h * COUT + nch * CH : h * COUT + (nch + 1) * CH,
                        ],
                        start=(h == 0),
                        stop=(h == HPC - 1),
                        skip_group_check=True,
                    )
            for gi in range(2):
                nch = g * 2 + gi
                if gi == 0:
                    nc.scalar.copy(osb[:, nch * CH : (nch + 1) * CH], psos[gi][:])
                else:
                    nc.vector.tensor_copy(
                        osb[:, nch * CH : (nch + 1) * CH], psos[gi][:]
                    )
            if g == 1:
                nc.sync.dma_start(
                    part[b_ * T + m * 128 : b_ * T + (m + 1) * 128, :], osb[:]
                )
            return True

        def flush_norm(keep):
            while len(normq) > keep:
                Rf_, psy_, ysb_sl = normq.pop(0)
                nc.vector.tensor_mul(ysb_sl, psy_[:], Rf_[:])

        for b in range(B):
            # ---- qkv projection + RoPE for this batch ----
            qrot = [rot_pool.tile([128, T], bf16, tag="rot", name=f"qrot{h}") for h in range(HPC)]
            krot = [rot_pool.tile([128, T], bf16, tag="rot", name=f"krot{h}") for h in range(HPC)]
            vsb = v_pool.tile([128, NTT * F], bf16, tag="v")
            for c in range(NCH):
                if b == 0 and c == 0:
                    xc = xc0
                else:
                    xc = xc_pool.tile([128, KB * CH], bf16, tag="xc")
                    gc = b * NCH + c
                    nc.sync.dma_start(
                        xc[:], xt[:, gc * KB * CH : (gc + 1) * KB * CH]
                    )
                for h in range(HPC):
                    for wsb, ccs, sss, dst in (
                        (wq_sb, ccq_sb, ssq_sb, qrot[h]),
                        (wk_sb, cck_sb, ssk_sb, krot[h]),
                    ):
                        ps = ps_a.tile([128, CH], f32, tag="psa")
                        for kb in range(KB):
                            nc.tensor.matmul(
                                ps[:],
                                wsb[:, kb * F + h * D : kb * F + (h + 1) * D],
                                xc[:, kb * CH : (kb + 1) * CH],
                                start=(kb == 0),
                                stop=(kb == KB - 1),
                            )
                        # RoPE: rot = ps * cc + halfswap(ps) * ss
                        sw = swap_pool.tile([128, CH], bf16, tag="swap")
                        nc.vector.tensor_copy(sw[0:64, :], ps[64:128, :])
                        nc.vector.tensor_copy(sw[64:128, :], ps[0:64, :])
                        A = a_pool.tile([128, CH], f32, tag="ropeA")
                        nc.vector.tensor_mul(
                            A[:], ps[:], ccs[:, c * CH : (c + 1) * CH]
                        )
                        Bt = a_pool.tile([128, CH], bf16, tag="ropeB")
                        nc.vector.tensor_mul(
                            Bt[:], sw[:], sss[:, c * CH : (c + 1) * CH]
                        )
                        nc.vector.tensor_add(
                            dst[:, c * CH : (c + 1) * CH], A[:], Bt[:]
                        )
                # V in [token part, feature free] layout: x-blocks stationary
                for tm in range(CH // 128):
                    psv = ps_b.tile([128, F], f32, tag="psb", name="psv")
                    for kb in range(KB):
                        nc.tensor.matmul(
                            psv[:],
                            xc[:, kb * CH + tm * 128 : kb * CH + tm * 128 + 128],
                            wv_sb[:, kb * F : (kb + 1) * F],
                            start=(kb == 0),
                            stop=(kb == KB - 1),
                        )
                    tt = c * (CH // 128) + tm
                    nc.scalar.copy(vsb[:, tt * F : (tt + 1) * F], psv[:])

            # ---- attention per head: k-block-outer over PAIRS of q-chunks,
            # so both live chunks stream against each krot/vsb stationary
            # back-to-back.  The softmax normalization is lagged: the gpsimd
            # partition_all_reduce (~2.9us) is issued at the chunk's
            # finalize, but the DVE recip+mul that depends on it is deferred
            # to the NEXT finalize point so the in-order DVE queue never
            # blocks waiting on gpsimd.
            ysb = [y_pool.tile([128, T], bf16, tag="y", name=f"ysb{h}") for h in range(HPC)]
            for h in range(HPC):
                for pair in ((0, 1), (2, 3)):
                    psy = {}
                    uacc = {}
                    for j in pair:
                        psy[j] = ps_b.tile([128, CH], f32, tag="psb", name=f"psy{j}")
                        uacc[j] = ua_pool.tile(
                            [128, CH], bf16, tag="uacc", name=f"uacc{j}"
                        )
                    U0s = {}
                    for kb in range((max(pair) + 1) * (CH // 128)):
                        live = [j for j in pair if kb <= j * (CH // 128) + 3]
                        Us = {}
                        # S^T matmuls for live chunks vs this k-stationary
                        for j in live:
                            c0 = max(0, kb * 128 - j * CH)
                            psS = ps_a.tile([128, CH], f32, tag="psa", name="psS")
                            nc.tensor.matmul(
                                psS[:, c0:CH],
                                krot[h][:, kb * 128 : (kb + 1) * 128],
                                qrot[h][:, j * CH + c0 : (j + 1) * CH],
                                start=True,
                                stop=True,
                            )
                            U = u_pool.tile([128, CH], bf16, tag="u")
                            nc.scalar.activation(U[:, c0:CH], psS[:, c0:CH], AF.Exp)
                            if kb * 128 >= j * CH:
                                # diagonal block: zero out k > q after exp
                                nc.vector.tensor_mul(
                                    U[:, c0 : c0 + 128],
                                    U[:, c0 : c0 + 128],
                                    triu_sb[:],
                                )
                            Us[j] = (U, c0)
                        # outproj filler: covers the exp latency of the last
                        # S before its y consumes it, and drains the queue
                        emit_unit()
                        if len(live) == 1:
                            emit_unit()
                        # y accumulation for live chunks vs vsb stationary.
                        # Reversed: the diagonal (lowest-j) chunk's U has the
                        # longest exp->mask chain, so consume it last.
                        for j in reversed(live):
                            U, c0 = Us[j]
                            nc.tensor.matmul(
                                psy[j][:, c0:CH],
                                vsb[:, kb * F + h * D : kb * F + (h + 1) * D],
                                U[:, c0:CH],
                                start=(kb == 0),
                                stop=(kb == (j + 1) * (CH // 128) - 1),
                                skip_group_check=True,
                            )
                        # denominator accumulation on DVE (off the PE).
                        # kb==0 blocks are held and merged into the kb==1
                        # add, saving a copy per chunk.
                        for j in live:
                            U, c0 = Us[j]
                            if kb == 0 and j == 0:
                                # chunk 0's kb==1 block is diagonal-trimmed
                                # (c0=128), so the merged add below would read
                                # garbage; plain copy instead
                                nc.vector.tensor_copy(uacc[j][:], U[:])
                            elif kb == 0:
                                U0s[j] = U
                            elif kb == 1 and j in U0s:
                                nc.vector.tensor_add(
                                    uacc[j][:], U0s.pop(j)[:], U[:]
                                )
                            else:
                                nc.vector.tensor_add(
                                    uacc[j][:, c0:CH],
                                    uacc[j][:, c0:CH],
                                    U[:, c0:CH],
                                )
                        # finalize a chunk whose last k-tile this was:
                        # issue its all_reduce, flush the previous chunk's
                        # deferred normalize, stage/promote outproj units
                        for j in live:
                            if kb != j * (CH // 128) + 3:
                                continue
                            # partition-reduce the accumulated exp sums with
                            # one cheap ones-matmul (213ns PE) instead of a
                            # gpsimd all_reduce (3.5us, and it starves DVE of
                            # SBUF bandwidth while running)
                            emit_unit()
                            psd = ps_c.tile([1, CH], f32, tag="psc", name="psd")
                            nc.tensor.matmul(
                                psd[:],
                                onesb_sb[:],
                                uacc[j][:],
                                start=True,
                                stop=True,
                                skip_group_check=True,
                            )
                            rr = rr_pool.tile([1, CH], f32, tag="rr", name="rr")
                            nc.vector.reciprocal_approx_fast(rr[:], psd[:])
                            Rf = r_pool.tile([128, CH], f32, tag="recip", name="rf")
                            nc.gpsimd.partition_broadcast(Rf[:], rr[:])
                            flush_norm(keep=0)
                            normq.append(
                                (Rf, psy[j], ysb[h][:, j * CH : (j + 1) * CH])
                            )
                            if h == HPC - 1:
                                unitq.extend(staging)
                                staging.clear()
                                for m in range(
                                    j * (CH // 128), (j + 1) * (CH // 128)
                                ):
                                    for g in range(2):
                                        staging.append((ysb, b, j, m, g))

        flush_norm(keep=0)
        unitq.extend(staging)
        staging.clear()
        while emit_unit():
            pass

    nc.compile()
    return nc


def make_host_inputs(x, cos, sin, Wqkv, Wout, H, n_cores):
    """Shard + precompute the per-core device input maps (numpy, host side)."""
    bf16 = ml_dtypes.bfloat16
    B, T, C = x.shape
    D = C // H
    HPC = H // n_cores
    COUT = Wout.shape[1]

    CH = _CH
    KB = C // 128
    # xt pre-tiled so each partition's per-chunk slice is contiguous in
    # DRAM: [p, global_chunk, kb, tok]
    xt = (
        x.reshape(B * T, C)
        .T.reshape(KB, 128, B * T // CH, CH)
        .transpose(1, 2, 0, 3)
        .reshape(128, -1)
    )
    xt = np.ascontiguousarray(xt).astype(bf16)

    def tile_w(w):
        # [C, F] -> [p, kb, F] contiguous
        Fw = w.shape[1]
        return np.ascontiguousarray(
            w.reshape(KB, 128, Fw).transpose(1, 0, 2).reshape(128, KB * Fw)
        ).astype(bf16)

    # deinterleave permutation within each head: [0,2,4,...,1,3,5,...]
    perm = np.concatenate([np.arange(0, D, 2), np.arange(1, D, 2)])
    Wq = Wqkv[:, 0:C].reshape(C, H, D)[:, :, perm]
    Wk = Wqkv[:, C : 2 * C].reshape(C, H, D)[:, :, perm]
    Wv = Wqkv[:, 2 * C : 3 * C].reshape(C, H, D)

    cosT = cos.T  # [D/2, T]
    CC = np.concatenate([cosT, cosT], axis=0)  # [D, T]
    SS = np.concatenate([-sin.T, sin.T], axis=0)
    scale = 1.0 / math.sqrt(D)
    ccq = (CC * scale).astype(bf16)
    ssq = (SS * scale).astype(bf16)
    cck = CC.astype(bf16)
    ssk = SS.astype(bf16)

    tri = np.triu(np.ones((128, 128), dtype=np.float32)).astype(bf16)
    onesb = np.ones((128, 1), dtype=np.float32).astype(bf16)

    in_maps = []
    for core in range(n_cores):
        hs = slice(core * HPC, (core + 1) * HPC)
        in_maps.append(
            {
                "xt": xt,
                "wq": tile_w(Wq[:, hs, :].reshape(C, HPC * D)),
                "wk": tile_w(Wk[:, hs, :].reshape(C, HPC * D)),
                "wv": tile_w(Wv[:, hs, :].reshape(C, HPC * D)),
                "wo": np.ascontiguousarray(
                    Wout[core * HPC * D : (core + 1) * HPC * D, :]
                    .reshape(HPC, 128, COUT)
                    .transpose(1, 0, 2)
                    .reshape(128, HPC * COUT)
                ).astype(bf16),
                "ccq": ccq,
                "ssq": ssq,
                "cck": cck,
                "ssk": ssk,
                "triu": tri,
                "onesb": onesb,
            }
        )
    return in_maps


_PROGRAM_CACHE = {}


def kernel(x, cos, sin, Wqkv, Wout):
    global LAST_RESULT
    from concourse.bass_utils import run_bass_kernel_spmd

    x = np.asarray(x, dtype=np.float32)
    cos = np.asarray(cos, dtype=np.float32)
    sin = np.asarray(sin, dtype=np.float32)
    Wqkv = np.asarray(Wqkv, dtype=np.float32)
    Wout = np.asarray(Wout, dtype=np.float32)

    B, T, C = x.shape
    H = _H
    COUT = Wout.shape[1]
    n_cores = 8
    HPC = H // n_cores

    key = (B, T, C, COUT, HPC, n_cores)
    if key not in _PROGRAM_CACHE:
        _PROGRAM_CACHE[key] = build_program(B, T, C, COUT, HPC, n_cores)
    nc = _PROGRAM_CACHE[key]

    in_maps = make_host_inputs(x, cos, sin, Wqkv, Wout, H, n_cores)
    res = run_bass_kernel_spmd(
        nc, in_maps, core_ids=list(range(n_cores)), trace=TRACE
    )
    LAST_RESULT = res

    out = np.zeros((B * T, COUT), dtype=np.float32)
    for r in res.results:
        out += np.asarray(r["part"], dtype=np.float32)
    return out.reshape(B, T, COUT)



# revision 4
# speedup vs baseline: 1.0662x; 1.0662x over previous
"""Trainium2 Bass kernel for causal self-attention (B=4, T=2048, C=2048, H=16).

Sharding: 16 heads across 8 cores (2 heads/core), tensor-parallel column split
of Wqkv and row split of Wout; host sums the 8 row-parallel partial outputs.

Device-side layout strategy (everything "transposed", token index in the free
dimension), which makes every matmul land in its natural layout with zero
on-chip transposes:
  qT/kT   [D=128 part, T free]   = Wq_h^T @ x  (m-tile of the qkv projection)
  V       [T part, D free]       computed with x-blocks as the stationary operand
  S^T     [k part, q free]       = kT-block^T-contraction (lhsT=kT blk, rhs=qT)
  U = exp(S^T), causal blocks skipped entirely, diagonal blocks masked via a
      triangular 0/1 multiply after exp
  y^T     [D part, q free]       = sum_kb V_blk.T @ U_blk  (PSUM accumulate)
  denom   [1, q]  uacc summed across partitions with one cheap ones-matmul
  y_norm  = y^T * reciprocal(denom broadcast)
  partial [tokens part, Cout]    lhsT = y^T block, rhs = Wout rows

Schedule: the projection of batch b+1 is software-pipelined INTO the attention
of batch b.  Attention alone leaves the PE ~20% idle (waiting on the
ACT-exp chain) and the idle dips also drop the HAM PE clock from 2.4 to
1.4 GHz, slowing the matmuls that do run.  Projection work is pure dense PE
with almost no ACT/DVE load, so interleaving the two keeps the PE saturated
for the whole kernel and spreads the elementwise work evenly:
  - proj(b+1) is emitted as ~48 "steps" (half a qk-projection chain, or one
    V-projection chain, ~1.7us of PE each); one step is emitted per
    attention k-block, between the S matmuls and the y matmuls, exactly
    where the exp latency needs covering.
  - out-projection micro-units (2 wo-chunks x HPC matmuls) remain a second
    filler source, emitted once per k-block and at finalizes.
  - RoPE epilogues (swap copies on ACT, muls/adds on DVE) lag their chain by
    one step so no engine queue head-of-line blocks on the PE.
  - out-proj PSUM->SBUF copies rotate over (gpsimd,gpsimd) / (ACT,DVE) per
    unit to balance the three elementwise engines.
  - softmax normalize (recip+mul) is flushed one k-block after its finalize
    so the DVE never waits on the gpsimd broadcast in-queue.
  - the final drain (last batch's out-proj units) rotates PSUM banks across
    the 3 then-idle pools so units pipeline instead of serializing.

RoPE is applied in [D, T] layout: the q/k columns of Wqkv are permuted on the
host so rotation pairs land at partitions (i, i+64) ("deinterleaved"); the
half-swap is two 64-partition ACT copies, then 3 elementwise ops against
host-precomputed [128, T] cos/sin tables (the 1/sqrt(D) scale is folded into
the q tables). Scores are invariant to any fixed permutation of head dims
applied to both q and k, so the reference is reproduced exactly.
"""

import math

import numpy as np
import ml_dtypes

# Module-level knobs (test harness may set TRACE=True to capture a profile).
TRACE = False
LAST_RESULT = None  # BassKernelResults of the last run (for profiling)

_B, _T, _C, _H = 4, 2048, 2048, 16
_D = 128
_CH = 512  # free-dim chunk (one PSUM bank of fp32)


def build_program(B, T, C, COUT, HPC, n_cores=8):
    """Build the per-core Bass/Tile program (identical on all cores)."""
    import concourse.bass as bass  # noqa: F401
    import concourse.mybir as mybir
    import concourse.tile as tile
    from concourse import bacc
    from concourse import bass_isa
    from contextlib import ExitStack

    dt = mybir.dt
    f32, bf16 = dt.float32, dt.bfloat16
    D, CH = _D, _CH
    KB = C // 128          # contraction blocks for the projections
    NCH = T // CH          # 512-token chunks per batch (also q-chunks)
    NTT = T // 128         # 128-token tiles per batch (also k-tiles)
    BT = B * T
    F = HPC * D            # per-core head feature width
    PIECE = 4              # xc kb-blocks per DMA piece
    NP = KB // PIECE       # pieces per chunk
    AF = mybir.ActivationFunctionType

    nc = bacc.Bacc(
        "TRN2", target_bir_lowering=False, debug=False, num_devices=n_cores
    )

    # All weight/activation inputs are pre-tiled on the host so that each
    # SBUF partition's slice is one fully contiguous DRAM run: the DMA then
    # moves 8-16KB descriptors instead of 0.5-1KB ones (~8x faster loads).
    xt = nc.dram_tensor("xt", [128, B * NCH * KB * CH], bf16, kind="ExternalInput").ap()
    wq = nc.dram_tensor("wq", [128, KB * F], bf16, kind="ExternalInput").ap()
    wk = nc.dram_tensor("wk", [128, KB * F], bf16, kind="ExternalInput").ap()
    wv = nc.dram_tensor("wv", [128, KB * F], bf16, kind="ExternalInput").ap()
    wo = nc.dram_tensor("wo", [128, HPC * COUT], bf16, kind="ExternalInput").ap()
    ccq = nc.dram_tensor("ccq", [128, T], bf16, kind="ExternalInput").ap()
    ssq = nc.dram_tensor("ssq", [128, T], bf16, kind="ExternalInput").ap()
    cck = nc.dram_tensor("cck", [128, T], bf16, kind="ExternalInput").ap()
    ssk = nc.dram_tensor("ssk", [128, T], bf16, kind="ExternalInput").ap()
    triu = nc.dram_tensor("triu", [128, 128], bf16, kind="ExternalInput").ap()
    onesb = nc.dram_tensor("onesb", [128, 1], bf16, kind="ExternalInput").ap()
    part = nc.dram_tensor("part", [BT, COUT], bf16, kind="ExternalOutput").ap()

    with ExitStack() as ctx:
        tc = ctx.enter_context(tile.TileContext(nc))
        const_pool = ctx.enter_context(tc.tile_pool(name="const", bufs=1))
        xc_pool = ctx.enter_context(tc.tile_pool(name="xc", bufs=2 * NP))
        rot_pool = ctx.enter_context(tc.tile_pool(name="rot", bufs=4 * HPC))
        swap_pool = ctx.enter_context(tc.tile_pool(name="swap", bufs=3))
        a_pool = ctx.enter_context(tc.tile_pool(name="ropeA", bufs=5))
        v_pool = ctx.enter_context(tc.tile_pool(name="vsb", bufs=2))
        u_pool = ctx.enter_context(tc.tile_pool(name="u", bufs=12))
        ua_pool = ctx.enter_context(tc.tile_pool(name="uacc", bufs=6))
        r_pool = ctx.enter_context(tc.tile_pool(name="recip", bufs=4))
        rr_pool = ctx.enter_context(tc.tile_pool(name="rr", bufs=3))
        y_pool = ctx.enter_context(tc.tile_pool(name="y", bufs=2 * HPC))
        o_pool = ctx.enter_context(tc.tile_pool(name="osb", bufs=3))
        # PSUM banks (8 total):
        #   ps_s (2): S^T tiles in attention
        #   ps_y (2): psy accumulators (one pair of live q-chunks)
        #   ps_p (2): interleaved projection chains (qk and V alternate)
        #   ps_o (2): out-projection accumulators
        ps_s = ctx.enter_context(tc.tile_pool(name="pss", bufs=2, space="PSUM"))
        ps_y = ctx.enter_context(tc.tile_pool(name="psy", bufs=2, space="PSUM"))
        ps_p = ctx.enter_context(tc.tile_pool(name="psp", bufs=2, space="PSUM"))
        ps_o = ctx.enter_context(tc.tile_pool(name="pso", bufs=2, space="PSUM"))

        # ---- persistent constants ----
        # Preload split across the two hardware DGE queues (sync + scalar):
        # sync carries the first-chain critical inputs (wq, wk, q-tables);
        # scalar carries the first xc chunk + everything needed later.
        wq_sb = const_pool.tile([128, KB * F], bf16, tag="wq")
        wk_sb = const_pool.tile([128, KB * F], bf16, tag="wk")
        nc.sync.dma_start(wq_sb[:], wq)

        xc_tiles = {}  # (b, c) -> list of NP piece tiles

        def issue_xc_dma(b, c, eng=None):
            eng = eng or nc.sync
            gc = b * NCH + c
            pieces = []
            for p in range(NP):
                t = xc_pool.tile([128, PIECE * CH], bf16, tag="xc")
                base = gc * KB * CH + p * PIECE * CH
                eng.dma_start(t[:], xt[:, base : base + PIECE * CH])
                pieces.append(t)
            xc_tiles[(b, c)] = pieces

        issue_xc_dma(0, 0, nc.scalar)
        nc.sync.dma_start(wk_sb[:], wk)
        ccq_sb = const_pool.tile([128, T], bf16, tag="ccq")
        ssq_sb = const_pool.tile([128, T], bf16, tag="ssq")
        cck_sb = const_pool.tile([128, T], bf16, tag="cck")
        ssk_sb = const_pool.tile([128, T], bf16, tag="ssk")
        for csb, cdr in ((ccq_sb, ccq), (ssq_sb, ssq)):
            nc.sync.dma_start(csb[:], cdr)
        wv_sb = const_pool.tile([128, KB * F], bf16, tag="wv")
        nc.scalar.dma_start(wv_sb[:], wv)
        for csb, cdr in ((cck_sb, cck), (ssk_sb, ssk)):
            nc.scalar.dma_start(csb[:], cdr)
        triu_sb = const_pool.tile([128, 128], bf16, tag="triu")
        nc.sync.dma_start(triu_sb[:], triu)
        onesb_sb = const_pool.tile([128, 1], bf16, tag="onesb")
        nc.sync.dma_start(onesb_sb[:], onesb)
        wo_sb = const_pool.tile([128, HPC * COUT], bf16, tag="wo")
        nc.scalar.dma_start(wo_sb[:], wo)

        # ---- PE warm-up: keep the HAM clock gate busy while the initial
        # DMA wave streams in, so the first real matmuls run at 2.4 GHz ----
        junk = const_pool.tile([128, 128], bf16, tag="warmjunk")
        nc.vector.memset(junk[:], 0)
        psw = ps_s.tile([128, 128], f32, tag="pss", name="pswarm")
        for _ in range(64):
            nc.tensor.matmul(
                psw[:], junk[:], junk[:], start=True, stop=True,
                skip_group_check=True,
            )

        # ---- projection emitted as a resumable generator of ~1.7us steps ----
        def proj_gen(b, qrot, krot, vsb):
            pend = [None]  # epilogue of the previous chain, lagged one step

            def flush():
                if pend[0] is not None:
                    pend[0]()
                    pend[0] = None

            for c in range(NCH):
                if c + 1 < NCH:
                    issue_xc_dma(b, c + 1)
                pieces = xc_tiles[(b, c)]
                for h in range(HPC):
                    for wsb, ccs, sss, dst in (
                        (wq_sb, ccq_sb, ssq_sb, qrot[h]),
                        (wk_sb, cck_sb, ssk_sb, krot[h]),
                    ):
                        ps = ps_p.tile([128, CH], f32, tag="psp")
                        for half in range(2):
                            for kb in range(half * 8, half * 8 + 8):
                                pt = pieces[kb // PIECE]
                                sub = kb % PIECE
                                nc.tensor.matmul(
                                    ps[:],
                                    wsb[:, kb * F + h * D : kb * F + (h + 1) * D],
                                    pt[:, sub * CH : (sub + 1) * CH],
                                    start=(kb == 0),
                                    stop=(kb == KB - 1),
                                    skip_group_check=True,
                                )
                            if half == 0:
                                yield
                        flush()

                        def rope_ep(ps=ps, ccs=ccs, sss=sss, dst=dst, c=c):
                            # RoPE: rot = ps * cc + halfswap(ps) * ss
                            sw = swap_pool.tile([128, CH], bf16, tag="swap")
                            nc.scalar.copy(sw[0:64, :], ps[64:128, :])
                            nc.scalar.copy(sw[64:128, :], ps[0:64, :])
                            A = a_pool.tile([128, CH], bf16, tag="ropeA")
                            nc.vector.tensor_mul(
                                A[:], ps[:], ccs[:, c * CH : (c + 1) * CH]
                            )
                            Bt = a_pool.tile([128, CH], bf16, tag="ropeB")
                            nc.vector.tensor_mul(
                                Bt[:], sw[:], sss[:, c * CH : (c + 1) * CH]
                            )
                            nc.vector.tensor_add(
                                dst[:, c * CH : (c + 1) * CH], A[:], Bt[:]
                            )

                        pend[0] = rope_ep
                        yield
                # V in [token part, feature free] layout: x-blocks stationary
                for tm in range(CH // 128):
                    psv = ps_p.tile([128, F], f32, tag="psp", name="psv")
                    for kb in range(KB):
                        pt = pieces[kb // PIECE]
                        sub = kb % PIECE
                        nc.tensor.matmul(
                            psv[:],
                            pt[:, sub * CH + tm * 128 : sub * CH + tm * 128 + 128],
                            wv_sb[:, kb * F : (kb + 1) * F],
                            start=(kb == 0),
                            stop=(kb == KB - 1),
                            skip_group_check=True,
                        )
                    flush()
                    tt = c * (CH // 128) + tm

                    def v_ep(psv=psv, tt=tt):
                        nc.scalar.copy(vsb[:, tt * F : (tt + 1) * F], psv[:])

                    pend[0] = v_ep
                    yield
            flush()

        # Out-projection micro-units (one m-tile half: 2 wo-chunks x HPC
        # matmuls = 4x512 streamed columns), interleaved into the attention
        # stream at per-k-block granularity as a second filler source.
        unitq = []
        staging = []
        osb_live = {}  # (b, j, m) -> osb tile awaiting its second half
        normq = []  # deferred (Dall, psy, ysb_slice) recip+mul, lagged 1 k-block
        ucount = [0]

        def emit_unit(pools=None):
            if not unitq:
                return False
            ysb_, b_, j_, m, g = unitq.pop(0)
            key = (b_, j_, m)
            if g == 0:
                osb = o_pool.tile([128, COUT], bf16, tag="osb", name="osb")
                osb_live[key] = osb
            else:
                osb = osb_live.pop(key)
            pool = pools[ucount[0] % len(pools)] if pools else ps_o
            psos = [
                pool.tile([128, CH], f32, tag=pool.name, name="psot")
                for _ in range(2)
            ]
            for h in range(HPC):
                for gi in range(2):
                    nch = g * 2 + gi
                    nc.tensor.matmul(
                        psos[gi][:],
                        ysb_[h][:, m * 128 : (m + 1) * 128],
                        wo_sb[
                            :,
                            h * COUT + nch * CH : h * COUT + (nch + 1) * CH,
                        ],
                        start=(h == 0),
                        stop=(h == HPC - 1),
                        skip_group_check=True,
                    )
            # copy-engine rotation: (ACT, DVE) / (DVE, ACT) per unit to even
            # the two queues (GPSIMD cannot read PSUM)
            flip = ucount[0] % 2
            ucount[0] += 1
            for gi in range(2):
                nch = g * 2 + gi
                dst = osb[:, nch * CH : (nch + 1) * CH]
                if gi == flip:
                    nc.scalar.copy(dst, psos[gi][:])
                else:
                    nc.vector.tensor_copy(dst, psos[gi][:])
            if g == 1:
                nc.sync.dma_start(
                    part[b_ * T + m * 128 : b_ * T + (m + 1) * 128, :], osb[:]
                )
            return True

        def flush_norm():
            while normq:
                Rf_, psy_, ysb_sl = normq.pop(0)
                nc.vector.tensor_mul(ysb_sl, psy_[:], Rf_[:])

        def alloc_rot(b):
            qrot = [
                rot_pool.tile([128, T], bf16, tag="rot", name=f"qrot{b}_{h}")
                for h in range(HPC)
            ]
            krot = [
                rot_pool.tile([128, T], bf16, tag="rot", name=f"krot{b}_{h}")
                for h in range(HPC)
            ]
            vsb = v_pool.tile([128, NTT * F], bf16, tag="v")
            return qrot, krot, vsb

        # batch 0's projection runs densely up front (PE-bound anyway).
        cur = alloc_rot(0)
        for _ in proj_gen(0, *cur):
            pass

        for b in range(B):
            qrot, krot, vsb = cur
            if b + 1 < B:
                issue_xc_dma(b + 1, 0)
                nxt = alloc_rot(b + 1)
                pg = proj_gen(b + 1, *nxt)
            else:
                nxt = None
                pg = None

            def step_proj():
                nonlocal pg
                if pg is None:
                    return False
                try:
                    next(pg)
                    return True
                except StopIteration:
                    pg = None
                    return False

            # ---- attention per head: k-block-outer over PAIRS of q-chunks,
            # so both live chunks stream against each krot/vsb stationary
            # back-to-back.
            ysb = [
                y_pool.tile([128, T], bf16, tag="y", name=f"ysb{b}_{h}")
                for h in range(HPC)
            ]
            for h in range(HPC):
                for pair in ((0, 1), (2, 3)):
                    psy = {}
                    uacc = {}
                    for j in pair:
                        psy[j] = ps_y.tile([128, CH], f32, tag="psy", name=f"psyt{j}")
                        uacc[j] = ua_pool.tile(
                            [128, CH], bf16, tag="uacc", name=f"uacc{j}"
                        )
                    U0s = {}
                    for kb in range((max(pair) + 1) * (CH // 128)):
                        flush_norm()
                        live = [j for j in pair if kb <= j * (CH // 128) + 3]
                        Us = {}
                        # S^T matmuls for live chunks vs this k-stationary
                        for j in live:
                            c0 = max(0, kb * 128 - j * CH)
                            psS = ps_s.tile([128, CH], f32, tag="pss", name="psS")
                            nc.tensor.matmul(
                                psS[:, c0:CH],
                                krot[h][:, kb * 128 : (kb + 1) * 128],
                                qrot[h][:, j * CH + c0 : (j + 1) * CH],
                                start=True,
                                stop=True,
                                skip_group_check=True,
                            )
                            U = u_pool.tile([128, CH], bf16, tag="u")
                            nc.scalar.activation(U[:, c0:CH], psS[:, c0:CH], AF.Exp)
                            if kb * 128 >= j * CH:
                                # diagonal block: zero out k > q after exp
                                nc.vector.tensor_mul(
                                    U[:, c0 : c0 + 128],
                                    U[:, c0 : c0 + 128],
                                    triu_sb[:],
                                )
                            Us[j] = (U, c0)
                        # fillers: one proj step (big) + one outproj unit,
                        # covering the exp latency before y consumes U
                        step_proj()
                        emit_unit()
                        if len(live) == 1:
                            if not step_proj():
                                emit_unit()
                        # y accumulation for live chunks vs vsb stationary.
                        # Reversed: the diagonal (lowest-j) chunk's U has the
                        # longest exp->mask chain, so consume it last.  At
                        # kb==0 use forward order so the fresh psy banks'
                        # WAR on the previous pair's deferred normalize has
                        # an extra few hundred ns to clear.
                        order = list(live) if kb == 0 else list(reversed(live))
                        for j in order:
                            U, c0 = Us[j]
                            nc.tensor.matmul(
                                psy[j][:, c0:CH],
                                vsb[:, kb * F + h * D : kb * F + (h + 1) * D],
                                U[:, c0:CH],
                                start=(kb == 0),
                                stop=(kb == (j + 1) * (CH // 128) - 1),
                                skip_group_check=True,
                            )
                        # denominator accumulation on DVE (off the PE).
                        # kb==0 blocks are held and merged into the kb==1
                        # add, saving a copy per chunk.
                        for j in live:
                            U, c0 = Us[j]
                            if kb == 0 and j == 0:
                                # chunk 0's kb==1 block is diagonal-trimmed
                                # (c0=128), so the merged add below would read
                                # garbage; plain copy instead
                                nc.vector.tensor_copy(uacc[j][:], U[:])
                            elif kb == 0:
                                U0s[j] = U
                            elif kb == 1 and j in U0s:
                                nc.vector.tensor_add(
                                    uacc[j][:], U0s.pop(j)[:], U[:]
                                )
                            else:
                                nc.vector.tensor_add(
                                    uacc[j][:, c0:CH],
                                    uacc[j][:, c0:CH],
                                    U[:, c0:CH],
                                )
                        # finalize a chunk whose last k-tile this was:
                        # partition-reduce the exp sums, reciprocal,
                        # broadcast; the dependent normalize is flushed at
                        # the NEXT k-block so the DVE queue never waits.
                        for j in live:
                            if kb != j * (CH // 128) + 3:
                                continue
                            emit_unit()
                            psd = ps_o.tile([1, CH], f32, tag="pso", name="psd")
                            nc.tensor.matmul(
                                psd[:],
                                onesb_sb[:],
                                uacc[j][:],
                                start=True,
                                stop=True,
                                skip_group_check=True,
                            )
                            rr = rr_pool.tile([1, CH], f32, tag="rr", name="rr")
                            nc.vector.reciprocal_approx_fast(rr[:], psd[:])
                            Rf = r_pool.tile([128, CH], f32, tag="recip", name="rf")
                            nc.gpsimd.partition_broadcast(Rf[:], rr[:])
                            normq.append(
                                (Rf, psy[j], ysb[h][:, j * CH : (j + 1) * CH])
                            )
                            if h == HPC - 1:
                                unitq.extend(staging)
                                staging.clear()
                                for m in range(
                                    j * (CH // 128), (j + 1) * (CH // 128)
                                ):
                                    for g in range(2):
                                        staging.append((ysb, b, j, m, g))
            # safety: drain any proj steps not consumed as filler
            while step_proj():
                pass
            cur = nxt

        flush_norm()
        unitq.extend(staging)
        staging.clear()
        # final drain: rotate PSUM across the three now-idle pools so units
        # pipeline 3-deep instead of serializing on ps_o bank turnaround
        while emit_unit(pools=[ps_o, ps_y, ps_s]):
            pass

    nc.compile()
    return nc


def make_host_inputs(x, cos, sin, Wqkv, Wout, H, n_cores):
    """Shard + precompute the per-core device input maps (numpy, host side)."""
    bf16 = ml_dtypes.bfloat16
    B, T, C = x.shape
    D = C // H
    HPC = H // n_cores
    COUT = Wout.shape[1]

    CH = _CH
    KB = C // 128
    # xt pre-tiled so each partition's per-chunk slice is contiguous in
    # DRAM: [p, global_chunk, kb, tok]
    xt = (
        x.reshape(B * T, C)
        .T.reshape(KB, 128, B * T // CH, CH)
        .transpose(1, 2, 0, 3)
        .reshape(128, -1)
    )
    xt = np.ascontiguousarray(xt).astype(bf16)

    def tile_w(w):
        # [C, F] -> [p, kb, F] contiguous
        Fw = w.shape[1]
        return np.ascontiguousarray(
            w.reshape(KB, 128, Fw).transpose(1, 0, 2).reshape(128, KB * Fw)
        ).astype(bf16)

    # deinterleave permutation within each head: [0,2,4,...,1,3,5,...]
    perm = np.concatenate([np.arange(0, D, 2), np.arange(1, D, 2)])
    Wq = Wqkv[:, 0:C].reshape(C, H, D)[:, :, perm]
    Wk = Wqkv[:, C : 2 * C].reshape(C, H, D)[:, :, perm]
    Wv = Wqkv[:, 2 * C : 3 * C].reshape(C, H, D)

    cosT = cos.T  # [D/2, T]
    CC = np.concatenate([cosT, cosT], axis=0)  # [D, T]
    SS = np.concatenate([-sin.T, sin.T], axis=0)
    scale = 1.0 / math.sqrt(D)
    ccq = (CC * scale).astype(bf16)
    ssq = (SS * scale).astype(bf16)
    cck = CC.astype(bf16)
    ssk = SS.astype(bf16)

    tri = np.triu(np.ones((128, 128), dtype=np.float32)).astype(bf16)
    onesb = np.ones((128, 1), dtype=np.float32).astype(bf16)

    in_maps = []
    for core in range(n_cores):
        hs = slice(core * HPC, (core + 1) * HPC)
        in_maps.append(
            {
                "xt": xt,
                "wq": tile_w(Wq[:, hs, :].reshape(C, HPC * D)),
                "wk": tile_w(Wk[:, hs, :].reshape(C, HPC * D)),
                "wv": tile_w(Wv[:, hs, :].reshape(C, HPC * D)),
                "wo": np.ascontiguousarray(
                    Wout[core * HPC * D : (core + 1) * HPC * D, :]
                    .reshape(HPC, 128, COUT)
                    .transpose(1, 0, 2)
                    .reshape(128, HPC * COUT)
                ).astype(bf16),
                "ccq": ccq,
                "ssq": ssq,
                "cck": cck,
                "ssk": ssk,
                "triu": tri,
                "onesb": onesb,
            }
        )
    return in_maps


_PROGRAM_CACHE = {}


def kernel(x, cos, sin, Wqkv, Wout):
    global LAST_RESULT
    from concourse.bass_utils import run_bass_kernel_spmd

    x = np.asarray(x, dtype=np.float32)
    cos = np.asarray(cos, dtype=np.float32)
    sin = np.asarray(sin, dtype=np.float32)
    Wqkv = np.asarray(Wqkv, dtype=np.float32)
    Wout = np.asarray(Wout, dtype=np.float32)

    B, T, C = x.shape
    H = _H
    COUT = Wout.shape[1]
    n_cores = 8
    HPC = H // n_cores

    key = (B, T, C, COUT, HPC, n_cores)
    if key not in _PROGRAM_CACHE:
        _PROGRAM_CACHE[key] = build_program(B, T, C, COUT, HPC, n_cores)
    nc = _PROGRAM_CACHE[key]

    in_maps = make_host_inputs(x, cos, sin, Wqkv, Wout, H, n_cores)
    res = run_bass_kernel_spmd(
        nc, in_maps, core_ids=list(range(n_cores)), trace=TRACE
    )
    LAST_RESULT = res

    out = np.zeros((B * T, COUT), dtype=np.float32)
    for r in res.results:
        out += np.asarray(r["part"], dtype=np.float32)
    return out.reshape(B, T, COUT)
